# revision 14
# baseline (speedup 1.0000x reference)
"""Routed-LoRA linear layer (moe_routing) on 8 trn2 NeuronCores.

Math (per token t):
  out[t, :] = W @ x[t] + b + 2.0 * sum_n mask[n, t] * (B_n @ (A_n @ x[t]))

Strategy:
  - Data-parallel over B*T = 65536 tokens: 8192 tokens per core.
  - Streaming operands are marshaled to bf16 host-side: halves HBM traffic
    and SBUF footprint; output error ~3e-3 relative, well inside the 2e-2
    gate. PSUM accumulation stays fp32.
  - Host-side transposes give the device contiguous, partition-friendly
    layouts only:
      xt   [D_IN, TOK]   = x-shard transposed (contraction dim major)
      wt   [D_IN, D_OUT] = W.T
      at   [P, KC*NR]    = fused-A.T, pre-packed per partition
      btr  [NR, D_OUT]   = fused-B.T
      mtok [P, G, N]     = per-token routing mask, token-partition layout
  - The LoRA s = A@x projection rides the main matmul's M-tiles as a third
    small N=64 matmul per contraction chunk (2 PE cyc/token instead of 8
    for a separate [NR,SUP]-layout pass), is masked on DVE with a stride-0
    rank-broadcast AP, PE-transposed back to rank-partition layout, and
    accumulated into the base matmul's PSUM bank as a 9th contraction
    chunk. Bias is added during the PSUM->SBUF copy; output is stored
    bf16 and upcast on host.
  - Supertile 0 runs k-outer across two q-tiles so the PE consumes each
    arriving (x-chunk, W-chunk) DMA pair slower than the stream delivers;
    per-128-token output stores keep the drain tail short.
"""

import numpy as np
import ml_dtypes

import concourse.bass as bass
from concourse import bacc
from concourse.masks import make_identity
import concourse.mybir as mybir
import concourse.tile as tile
from concourse.bass_utils import run_bass_kernel_spmd

N_CORES = 8
B, T = 8, 8192
D_IN = 1024
D_OUT = 1024
N_ADAPT, R = 4, 16
NR = N_ADAPT * R  # 64
SCALING = 32.0 / 16.0

TOK = B * T // N_CORES  # 8192 tokens per core
SUP = 512               # tokens per supertile
N_SUP = TOK // SUP      # 16
SUB = 128               # tokens per matmul M-tile
N_SUB = SUP // SUB      # 4
G = N_SUP * N_SUB       # 64 M-tiles per core
P = 128
KC = D_IN // P          # 8 contraction chunks
NB = D_OUT // 512       # 2 PSUM-bank column halves

F32 = mybir.dt.float32
BF16 = mybir.dt.bfloat16
NP_BF16 = ml_dtypes.bfloat16


def build_bass(xp_bufs=4, op_bufs=4, pso_bufs=4):
    nc = bacc.Bacc(
        "TRN2", target_bir_lowering=False, debug=False, num_devices=N_CORES
    )

    xt_d = nc.dram_tensor("xt", [D_IN, TOK], BF16, kind="ExternalInput")
    wt_d = nc.dram_tensor("wt", [D_IN, D_OUT], BF16, kind="ExternalInput")
    at_d = nc.dram_tensor("at", [P, KC * NR], BF16, kind="ExternalInput")
    bt_d = nc.dram_tensor("btr", [NR, D_OUT], BF16, kind="ExternalInput")
    bias_d = nc.dram_tensor("bias", [D_OUT], BF16, kind="ExternalInput")
    mtok_d = nc.dram_tensor("mtok", [P, G * N_ADAPT], BF16, kind="ExternalInput")
    out_d = nc.dram_tensor("out", [TOK, D_OUT], BF16, kind="ExternalOutput")

    xt_r = xt_d.ap().rearrange("(kc p) t -> p kc t", p=P)
    wt_r = wt_d.ap().rearrange("(kc p) n -> p kc n", p=P)
    out_r = out_d.ap().rearrange("(s q p) n -> s q p n", q=N_SUB, p=P)
    bias_bcast = bass.AP(
        tensor=bias_d, offset=0, ap=[[0, P], [1, D_OUT]]
    )

    with tile.TileContext(nc) as tc:
        with (
            tc.tile_pool(name="const", bufs=1) as const,
            tc.tile_pool(name="xp", bufs=xp_bufs) as xp,
            tc.tile_pool(name="smqp", bufs=2) as smqp,
            tc.tile_pool(name="smtp", bufs=2) as smtp,
            tc.tile_pool(name="op", bufs=op_bufs) as op,
            tc.tile_pool(name="pso", bufs=pso_bufs, space="PSUM") as pso,
            tc.tile_pool(name="psj", bufs=2, space="PSUM") as psj,
            tc.tile_pool(name="pst", bufs=2, space="PSUM") as pst,
        ):
            w_sb = const.tile([P, KC, D_OUT], BF16)
            a_sb = const.tile([P, KC, NR], BF16)
            bt_sb = const.tile([NR, D_OUT], BF16)
            b_sb = const.tile([P, D_OUT], BF16)
            m_sb = const.tile([P, G, N_ADAPT], BF16)
            ident = const.tile([P, P], BF16)
            make_identity(nc, ident[:])
            # Preload order matters for startup latency: the first matmuls
            # need a_sb + x0 chunk 0 (sync queue) and W chunk k in order
            # (scalar queue); everything else is needed later.
            for k in range(KC):
                nc.scalar.dma_start(out=w_sb[:, k, :], in_=wt_r[:, k, :])
            nc.scalar.dma_start(out=bt_sb[:], in_=bt_d.ap())
            nc.scalar.dma_start(out=b_sb[:], in_=bias_bcast)

            for s in range(N_SUP):
                t0 = s * SUP
                x_sb = xp.tile([P, KC, SUP], BF16, tag="x")
                if s == 0:
                    # chunked first load: consumers of chunk k can start as
                    # soon as chunk k lands instead of after the full load
                    nc.sync.dma_start(
                        out=x_sb[:, 0:2, :], in_=xt_r[:, 0:2, t0 : t0 + SUP]
                    )
                    nc.sync.dma_start(
                        out=a_sb[:],
                        in_=at_d.ap().rearrange("p (kc j) -> p kc j", kc=KC),
                    )
                    for k in range(2, KC, 2):
                        nc.sync.dma_start(
                            out=x_sb[:, k : k + 2, :],
                            in_=xt_r[:, k : k + 2, t0 : t0 + SUP],
                        )
                    nc.sync.dma_start(
                        out=m_sb[:], in_=mtok_d.ap().rearrange(
                            "p (g n) -> p g n", g=G
                        )
                    )
                else:
                    nc.sync.dma_start(
                        out=x_sb[:], in_=xt_r[:, :, t0 : t0 + SUP]
                    )

                # per-q-tile state.  PSUM tiles are padded to a full 2KB
                # bank: accumulation-group `start` clears the whole bank, so
                # a bank must never host two in-flight groups.
                sj = {}    # [P, 64] fp32 s-projection (PSUM, bank-padded)
                smq = {}   # [P, 64] bf16 masked s, token-partition (SBUF)
                smt = {}   # [64, 128] bf16 masked s.T via PE transpose
                smt_sb = {}

                def sj_block(q):
                    # s[tok, j] += x-chunk.T @ A-chunk, rides the M-tile
                    sj[q] = psj.tile([P, 512], F32, tag="sj", name=f"sj{q}")
                    for k in range(KC):
                        nc.tensor.matmul(
                            sj[q][:, :NR],
                            x_sb[:, k, q * SUB : (q + 1) * SUB],
                            a_sb[:, k, :],
                            start=(k == 0),
                            stop=(k == KC - 1),
                        )

                def mask_q(q):
                    # rank-broadcast the per-adapter mask along r via a
                    # stride-0 AP: j = n*R + r
                    smq[q] = smqp.tile([P, NR], BF16, tag="smq", name=f"smq{q}")
                    m_bc = (
                        m_sb[:, s * N_SUB + q, :]
                        .unsqueeze(2)
                        .broadcast_to((P, N_ADAPT, R))
                    )
                    nc.vector.tensor_mul(
                        smq[q][:].rearrange("p (n r) -> p n r", n=N_ADAPT),
                        sj[q][:, :NR].rearrange("p (n r) -> p n r", n=N_ADAPT),
                        m_bc,
                    )

                def transpose_q(q):
                    smt[q] = pst.tile([NR, 1024], BF16, tag="smt", name=f"smt{q}")
                    nc.tensor.transpose(
                        smt[q][:, :SUB], smq[q][:], ident[:]
                    )

                def copy_q(q):
                    smt_sb[q] = smtp.tile(
                        [NR, SUB], BF16, tag="smtsb", name=f"smtsb{q}"
                    )
                    nc.vector.tensor_copy(smt_sb[q][:], smt[q][:, :SUB])

                def main_half(q, n, o_ps_h, skip=False):
                    ts = q * SUB
                    nsl = slice(n * 512, (n + 1) * 512)
                    for k in range(KC):
                        nc.tensor.matmul(
                            o_ps_h[:],
                            x_sb[:, k, ts : ts + SUB],
                            w_sb[:, k, nsl],
                            start=(k == 0),
                            stop=False,
                            skip_group_check=skip,
                        )

                def lora_half(q, n, o_ps_h, skip=False):
                    nsl = slice(n * 512, (n + 1) * 512)
                    nc.tensor.matmul(
                        o_ps_h[:],
                        smt_sb[q][:],
                        bt_sb[:, nsl],
                        start=False,
                        stop=True,
                        skip_group_check=skip,
                    )
                    o_sb = op.tile([P, 512], BF16, tag="o")
                    nc.vector.tensor_add(o_sb[:], o_ps_h[:], b_sb[:, nsl])
                    nc.scalar.dma_start(
                        out=out_r[s, q][:, nsl], in_=o_sb[:]
                    )

                if s == 0:
                    # Startup: k-outer across q0/q1 mains so each arriving
                    # (x-chunk, W-chunk) DMA pair unlocks PE work faster
                    # than the serialized preload stream delivers it.
                    ph01 = {}
                    for q in (0, 1):
                        for n in range(NB):
                            ph01[q, n] = pso.tile(
                                [P, 512], F32, tag="ops", name=f"ops01_{q}_{n}"
                            )
                    sj[0] = psj.tile([P, 512], F32, tag="sj", name="sj0a")
                    sj[1] = psj.tile([P, 512], F32, tag="sj", name="sj1a")
                    for k in range(KC):
                        for q in (0, 1):
                            for n in range(NB):
                                nsl = slice(n * 512, (n + 1) * 512)
                                nc.tensor.matmul(
                                    ph01[q, n][:],
                                    x_sb[:, k, q * SUB : (q + 1) * SUB],
                                    w_sb[:, k, nsl],
                                    start=(k == 0),
                                    stop=False,
                                    skip_group_check=True,
                                )
                        for q in (0, 1):
                            # rides the k-outer on its own psj bank (group
                            # interleaving is safe across banks)
                            nc.tensor.matmul(
                                sj[q][:, :NR],
                                x_sb[:, k, q * SUB : (q + 1) * SUB],
                                a_sb[:, k, :],
                                start=(k == 0),
                                stop=(k == KC - 1),
                                skip_group_check=True,
                            )
                    for q in (0, 1):
                        mask_q(q)
                    for q in (0, 1):
                        transpose_q(q)
                        copy_q(q)
                    for q in (0, 1):
                        for n in range(NB):
                            lora_half(q, n, ph01[q, n], skip=True)
                    sj_block(2)
                    mask_q(2)
                    sj_block(3)
                    mask_q(3)
                    transpose_q(2)
                    copy_q(2)
                    transpose_q(3)
                    copy_q(3)
                    for q in (2, 3):
                        o_ps = {}
                        for n in range(NB):
                            o_ps[n] = pso.tile(
                                [P, 512], F32, tag="ops", name=f"ops0_{q}_{n}"
                            )
                            main_half(q, n, o_ps[n])
                        for n in range(NB):
                            lora_half(q, n, o_ps[n])
                else:
                    # Steady state: sj/transpose/copy for tile q run early,
                    # interleaved with the q-1/q main matmuls, so the LoRA-B
                    # matmul never waits on the DVE round trip.
                    o_ps = {}
                    sj_block(0)
                    mask_q(0)
                    o_ps[0, 0] = pso.tile([P, 512], F32, tag="ops", name="opsA")
                    main_half(0, 0, o_ps[0, 0])
                    sj_block(1)
                    mask_q(1)
                    transpose_q(0)
                    copy_q(0)
                    o_ps[0, 1] = pso.tile([P, 512], F32, tag="ops", name="opsB")
                    main_half(0, 1, o_ps[0, 1])
                    lora_half(0, 0, o_ps[0, 0])
                    lora_half(0, 1, o_ps[0, 1])
                    for q in (1, 2):
                        sj_block(q + 1)
                        mask_q(q + 1)
                        transpose_q(q)
                        copy_q(q)
                        for n in range(NB):
                            o_ps[q, n] = pso.tile(
                                [P, 512], F32, tag="ops", name=f"ops_{q}_{n}"
                            )
                            main_half(q, n, o_ps[q, n])
                        for n in range(NB):
                            lora_half(q, n, o_ps[q, n])
                    transpose_q(3)
                    copy_q(3)
                    for n in range(NB):
                        o_ps[3, n] = pso.tile(
                            [P, 512], F32, tag="ops", name=f"ops_3_{n}"
                        )
                        main_half(3, n, o_ps[3, n])
                    for n in range(NB):
                        lora_half(3, n, o_ps[3, n])

    nc.compile()
    return nc


_NC_CACHE = None


def _get_nc():
    global _NC_CACHE
    if _NC_CACHE is None:
        _NC_CACHE = build_bass()
    return _NC_CACHE


def make_in_maps(x, W, b, lora_A, lora_B, masks):
    x = np.ascontiguousarray(x, dtype=np.float32)
    W = np.ascontiguousarray(W, dtype=np.float32)
    b = np.ascontiguousarray(b, dtype=np.float32)
    lora_A = np.ascontiguousarray(lora_A, dtype=np.float32)
    lora_B = np.ascontiguousarray(lora_B, dtype=np.float32)
    masks = np.ascontiguousarray(masks, dtype=np.float32)

    x_flat = x.reshape(B * T, D_IN)
    A_flat = lora_A.reshape(NR, D_IN)
    B_flat = lora_B.transpose(1, 0, 2).reshape(D_OUT, NR)

    wt = np.ascontiguousarray(W.T.astype(NP_BF16))       # [D_IN, D_OUT]
    # packed [P, KC*NR]: per-partition contiguous 1KB rows (full DMA rate)
    at = np.ascontiguousarray(
        A_flat.T.astype(NP_BF16).reshape(KC, P, NR).transpose(1, 0, 2)
        .reshape(P, KC * NR)
    )
    btr = np.ascontiguousarray(B_flat.T.astype(NP_BF16))  # [NR, D_OUT]

    # per-token mask, token-partition layout [P, G*N_ADAPT]
    m_full = masks[..., 0].reshape(N_ADAPT, B * T) * np.float32(SCALING)

    in_maps = []
    for c in range(N_CORES):
        sl = slice(c * TOK, (c + 1) * TOK)
        mtok = np.ascontiguousarray(
            m_full[:, sl].T.astype(NP_BF16)             # [TOK, N]
            .reshape(G, P, N_ADAPT).transpose(1, 0, 2)  # [P, G, N]
            .reshape(P, G * N_ADAPT)
        )
        in_maps.append(
            {
                "xt": np.ascontiguousarray(x_flat[sl].astype(NP_BF16).T),
                "wt": wt,
                "at": at,
                "btr": btr,
                "bias": b.astype(NP_BF16),
                "mtok": mtok,
            }
        )
    return in_maps


def kernel(x, W, b, lora_A, lora_B, masks):
    nc = _get_nc()
    in_maps = make_in_maps(x, W, b, lora_A, lora_B, masks)
    res = run_bass_kernel_spmd(nc, in_maps, core_ids=list(range(N_CORES)))
    out = np.concatenate([r["out"] for r in res.results], axis=0)
    out = out.astype(np.float32).reshape(B, T, D_OUT)
    return out


# revision 15
# speedup vs baseline: 1.0309x; 1.0309x over previous
"""Routed-LoRA linear layer (moe_routing) on 8 trn2 NeuronCores.

Math (per token t):
  out[t, :] = W @ x[t] + b + 2.0 * sum_n mask[n, t] * (B_n @ (A_n @ x[t]))

Strategy:
  - Data-parallel over B*T = 65536 tokens: 8192 tokens per core.
  - Streaming operands are marshaled to bf16 host-side: halves HBM traffic
    and SBUF footprint; output error ~3e-3 relative, well inside the 2e-2
    gate. PSUM accumulation stays fp32.
  - Host-side transposes give the device contiguous, partition-friendly
    layouts only:
      xt   [D_IN, TOK]   = x-shard transposed (contraction dim major)
      wt   [D_IN, D_OUT] = W.T
      at   [P, KC*NR]    = fused-A.T, pre-packed per partition
      btr  [NR, D_OUT]   = fused-B.T
      mtok [P, G, N]     = per-token routing mask, token-partition layout
  - The LoRA s = A@x projection rides the main matmul's M-tiles as a third
    small N=64 matmul per contraction chunk (2 PE cyc/token instead of 8
    for a separate [NR,SUP]-layout pass), is masked on DVE with a stride-0
    rank-broadcast AP, PE-transposed back to rank-partition layout, and
    accumulated into the base matmul's PSUM bank as a 9th contraction
    chunk. Bias is added during the PSUM->SBUF copy; output is stored
    bf16 and upcast on host.
  - Supertile 0 runs k-outer across two q-tiles so the PE consumes each
    arriving (x-chunk, W-chunk) DMA pair slower than the stream delivers;
    per-128-token output stores keep the drain tail short.
"""

import numpy as np
import ml_dtypes

import concourse.bass as bass
from concourse import bacc
from concourse.masks import make_identity
import concourse.mybir as mybir
import concourse.tile as tile
from concourse.bass_utils import run_bass_kernel_spmd

N_CORES = 8
B, T = 8, 8192
D_IN = 1024
D_OUT = 1024
N_ADAPT, R = 4, 16
NR = N_ADAPT * R  # 64
SCALING = 32.0 / 16.0

TOK = B * T // N_CORES  # 8192 tokens per core
SUP = 512               # tokens per supertile
N_SUP = TOK // SUP      # 16
SUB = 128               # tokens per matmul M-tile
N_SUB = SUP // SUB      # 4
G = N_SUP * N_SUB       # 64 M-tiles per core
P = 128
KC = D_IN // P          # 8 contraction chunks
NB = D_OUT // 512       # 2 PSUM-bank column halves

F32 = mybir.dt.float32
BF16 = mybir.dt.bfloat16
NP_BF16 = ml_dtypes.bfloat16


def build_bass(xp_bufs=4, op_bufs=4, pso_bufs=4):
    nc = bacc.Bacc(
        "TRN2", target_bir_lowering=False, debug=False, num_devices=N_CORES
    )

    xt_d = nc.dram_tensor("xt", [D_IN, TOK], BF16, kind="ExternalInput")
    wt_d = nc.dram_tensor("wt", [D_IN, D_OUT], BF16, kind="ExternalInput")
    at_d = nc.dram_tensor("at", [P, KC * NR], BF16, kind="ExternalInput")
    bt_d = nc.dram_tensor("btr", [NR, D_OUT], BF16, kind="ExternalInput")
    bias_d = nc.dram_tensor("bias", [D_OUT], BF16, kind="ExternalInput")
    mtok_d = nc.dram_tensor("mtok", [P, G * N_ADAPT], BF16, kind="ExternalInput")
    out_d = nc.dram_tensor("out", [TOK, D_OUT], BF16, kind="ExternalOutput")

    xt_r = xt_d.ap().rearrange("(kc p) t -> p kc t", p=P)
    wt_r = wt_d.ap().rearrange("(kc p) n -> p kc n", p=P)
    out_r = out_d.ap().rearrange("(s q p) n -> s q p n", q=N_SUB, p=P)
    bias_bcast = bass.AP(
        tensor=bias_d, offset=0, ap=[[0, P], [1, D_OUT]]
    )

    with tile.TileContext(nc) as tc:
        with (
            tc.tile_pool(name="const", bufs=1) as const,
            tc.tile_pool(name="xp", bufs=xp_bufs) as xp,
            tc.tile_pool(name="smqp", bufs=2) as smqp,
            tc.tile_pool(name="smtp", bufs=2) as smtp,
            tc.tile_pool(name="op", bufs=op_bufs) as op,
            tc.tile_pool(name="pso", bufs=pso_bufs, space="PSUM") as pso,
            tc.tile_pool(name="psj", bufs=2, space="PSUM") as psj,
            tc.tile_pool(name="pst", bufs=2, space="PSUM") as pst,
        ):
            w_sb = const.tile([P, KC, D_OUT], BF16)
            a_sb = const.tile([P, KC, NR], BF16)
            bt_sb = const.tile([NR, D_OUT], BF16)
            b_sb = const.tile([P, D_OUT], BF16)
            m_sb = const.tile([P, G, N_ADAPT], BF16)
            ident = const.tile([P, P], BF16)
            make_identity(nc, ident[:])
            # Preload order matters for startup latency: the first matmuls
            # need a_sb + x0 chunk 0 (sync queue) and W chunk k in order
            # (scalar queue); everything else is needed later.
            nc.gpsimd.dma_start(
                out=a_sb[:],
                in_=at_d.ap().rearrange("p (kc j) -> p kc j", kc=KC),
            )
            nc.gpsimd.dma_start(
                out=m_sb[:],
                in_=mtok_d.ap().rearrange("p (g n) -> p g n", g=G),
            )
            for k in range(KC):
                nc.scalar.dma_start(out=w_sb[:, k, :], in_=wt_r[:, k, :])
            nc.scalar.dma_start(out=bt_sb[:], in_=bt_d.ap())
            nc.scalar.dma_start(out=b_sb[:], in_=bias_bcast)

            for s in range(N_SUP):
                t0 = s * SUP
                x_sb = xp.tile([P, KC, SUP], BF16, tag="x")
                if s == 0:
                    # chunked first load: consumers of chunk k can start as
                    # soon as chunk k lands instead of after the full load
                    for k in range(0, KC, 2):
                        nc.sync.dma_start(
                            out=x_sb[:, k : k + 2, :],
                            in_=xt_r[:, k : k + 2, t0 : t0 + SUP],
                        )
                else:
                    nc.sync.dma_start(
                        out=x_sb[:], in_=xt_r[:, :, t0 : t0 + SUP]
                    )

                # per-q-tile state.  PSUM tiles are padded to a full 2KB
                # bank: accumulation-group `start` clears the whole bank, so
                # a bank must never host two in-flight groups.
                sj = {}    # [P, 64] fp32 s-projection (PSUM, bank-padded)
                smq = {}   # [P, 64] bf16 masked s, token-partition (SBUF)
                smt = {}   # [64, 128] bf16 masked s.T via PE transpose
                smt_sb = {}

                def sj_block(q):
                    # s[tok, j] += x-chunk.T @ A-chunk, rides the M-tile
                    sj[q] = psj.tile([P, 512], F32, tag="sj", name=f"sj{q}")
                    for k in range(KC):
                        nc.tensor.matmul(
                            sj[q][:, :NR],
                            x_sb[:, k, q * SUB : (q + 1) * SUB],
                            a_sb[:, k, :],
                            start=(k == 0),
                            stop=(k == KC - 1),
                        )

                def mask_q(q):
                    # rank-broadcast the per-adapter mask along r via a
                    # stride-0 AP: j = n*R + r
                    smq[q] = smqp.tile([P, NR], BF16, tag="smq", name=f"smq{q}")
                    m_bc = (
                        m_sb[:, s * N_SUB + q, :]
                        .unsqueeze(2)
                        .broadcast_to((P, N_ADAPT, R))
                    )
                    nc.vector.tensor_mul(
                        smq[q][:].rearrange("p (n r) -> p n r", n=N_ADAPT),
                        sj[q][:, :NR].rearrange("p (n r) -> p n r", n=N_ADAPT),
                        m_bc,
                    )

                def transpose_q(q):
                    smt[q] = pst.tile([NR, 1024], BF16, tag="smt", name=f"smt{q}")
                    nc.tensor.transpose(
                        smt[q][:, :SUB], smq[q][:], ident[:]
                    )

                def copy_q(q):
                    smt_sb[q] = smtp.tile(
                        [NR, SUB], BF16, tag="smtsb", name=f"smtsb{q}"
                    )
                    nc.vector.tensor_copy(smt_sb[q][:], smt[q][:, :SUB])

                def main_half(q, n, o_ps_h, skip=False):
                    ts = q * SUB
                    nsl = slice(n * 512, (n + 1) * 512)
                    for k in range(KC):
                        nc.tensor.matmul(
                            o_ps_h[:],
                            x_sb[:, k, ts : ts + SUB],
                            w_sb[:, k, nsl],
                            start=(k == 0),
                            stop=False,
                            skip_group_check=skip,
                        )

                def lora_half(q, n, o_ps_h, skip=False):
                    nsl = slice(n * 512, (n + 1) * 512)
                    nc.tensor.matmul(
                        o_ps_h[:],
                        smt_sb[q][:],
                        bt_sb[:, nsl],
                        start=False,
                        stop=True,
                        skip_group_check=skip,
                    )
                    o_sb = op.tile([P, 512], BF16, tag="o")
                    nc.vector.tensor_add(o_sb[:], o_ps_h[:], b_sb[:, nsl])
                    nc.scalar.dma_start(
                        out=out_r[s, q][:, nsl], in_=o_sb[:]
                    )

                if s == 0:
                    # Startup: k-outer across q0/q1 mains so each arriving
                    # (x-chunk, W-chunk) DMA pair unlocks PE work faster
                    # than the serialized preload stream delivers it.
                    ph01 = {}
                    for q in (0, 1):
                        for n in range(NB):
                            ph01[q, n] = pso.tile(
                                [P, 512], F32, tag="ops", name=f"ops01_{q}_{n}"
                            )
                    for k in range(KC):
                        for q in (0, 1):
                            for n in range(NB):
                                nsl = slice(n * 512, (n + 1) * 512)
                                nc.tensor.matmul(
                                    ph01[q, n][:],
                                    x_sb[:, k, q * SUB : (q + 1) * SUB],
                                    w_sb[:, k, nsl],
                                    start=(k == 0),
                                    stop=False,
                                    skip_group_check=True,
                                )
                    for q in range(N_SUB):
                        sj_block(q)
                        mask_q(q)
                    for q in range(N_SUB):
                        transpose_q(q)
                        copy_q(q)
                    for q in (0, 1):
                        for n in range(NB):
                            lora_half(q, n, ph01[q, n], skip=True)
                    for q in (2, 3):
                        o_ps = {}
                        for n in range(NB):
                            o_ps[n] = pso.tile(
                                [P, 512], F32, tag="ops", name=f"ops0_{q}_{n}"
                            )
                            main_half(q, n, o_ps[n])
                        for n in range(NB):
                            lora_half(q, n, o_ps[n])
                else:
                    # Steady state: sj/transpose/copy for tile q run early,
                    # interleaved with the q-1/q main matmuls, so the LoRA-B
                    # matmul never waits on the DVE round trip.
                    o_ps = {}
                    sj_block(0)
                    mask_q(0)
                    o_ps[0, 0] = pso.tile([P, 512], F32, tag="ops", name="opsA")
                    main_half(0, 0, o_ps[0, 0])
                    sj_block(1)
                    mask_q(1)
                    transpose_q(0)
                    copy_q(0)
                    o_ps[0, 1] = pso.tile([P, 512], F32, tag="ops", name="opsB")
                    main_half(0, 1, o_ps[0, 1])
                    lora_half(0, 0, o_ps[0, 0])
                    lora_half(0, 1, o_ps[0, 1])
                    for q in (1, 2):
                        sj_block(q + 1)
                        mask_q(q + 1)
                        transpose_q(q)
                        copy_q(q)
                        for n in range(NB):
                            o_ps[q, n] = pso.tile(
                                [P, 512], F32, tag="ops", name=f"ops_{q}_{n}"
                            )
                            main_half(q, n, o_ps[q, n])
                        for n in range(NB):
                            lora_half(q, n, o_ps[q, n])
                    transpose_q(3)
                    copy_q(3)
                    for n in range(NB):
                        o_ps[3, n] = pso.tile(
                            [P, 512], F32, tag="ops", name=f"ops_3_{n}"
                        )
                        main_half(3, n, o_ps[3, n])
                    for n in range(NB):
                        lora_half(3, n, o_ps[3, n])

    nc.compile()
    return nc


_NC_CACHE = None


def _get_nc():
    global _NC_CACHE
    if _NC_CACHE is None:
        _NC_CACHE = build_bass()
    return _NC_CACHE


def make_in_maps(x, W, b, lora_A, lora_B, masks):
    x = np.ascontiguousarray(x, dtype=np.float32)
    W = np.ascontiguousarray(W, dtype=np.float32)
    b = np.ascontiguousarray(b, dtype=np.float32)
    lora_A = np.ascontiguousarray(lora_A, dtype=np.float32)
    lora_B = np.ascontiguousarray(lora_B, dtype=np.float32)
    masks = np.ascontiguousarray(masks, dtype=np.float32)

    x_flat = x.reshape(B * T, D_IN)
    A_flat = lora_A.reshape(NR, D_IN)
    B_flat = lora_B.transpose(1, 0, 2).reshape(D_OUT, NR)

    wt = np.ascontiguousarray(W.T.astype(NP_BF16))       # [D_IN, D_OUT]
    # packed [P, KC*NR]: per-partition contiguous 1KB rows (full DMA rate)
    at = np.ascontiguousarray(
        A_flat.T.astype(NP_BF16).reshape(KC, P, NR).transpose(1, 0, 2)
        .reshape(P, KC * NR)
    )
    btr = np.ascontiguousarray(B_flat.T.astype(NP_BF16))  # [NR, D_OUT]

    # per-token mask, token-partition layout [P, G*N_ADAPT]
    m_full = masks[..., 0].reshape(N_ADAPT, B * T) * np.float32(SCALING)

    in_maps = []
    for c in range(N_CORES):
        sl = slice(c * TOK, (c + 1) * TOK)
        mtok = np.ascontiguousarray(
            m_full[:, sl].T.astype(NP_BF16)             # [TOK, N]
            .reshape(G, P, N_ADAPT).transpose(1, 0, 2)  # [P, G, N]
            .reshape(P, G * N_ADAPT)
        )
        in_maps.append(
            {
                "xt": np.ascontiguousarray(x_flat[sl].astype(NP_BF16).T),
                "wt": wt,
                "at": at,
                "btr": btr,
                "bias": b.astype(NP_BF16),
                "mtok": mtok,
            }
        )
    return in_maps


def kernel(x, W, b, lora_A, lora_B, masks):
    nc = _get_nc()
    in_maps = make_in_maps(x, W, b, lora_A, lora_B, masks)
    res = run_bass_kernel_spmd(nc, in_maps, core_ids=list(range(N_CORES)))
    out = np.concatenate([r["out"] for r in res.results], axis=0)
    out = out.astype(np.float32).reshape(B, T, D_OUT)
    return out


# revision 16
# speedup vs baseline: 1.0373x; 1.0062x over previous
"""Routed-LoRA linear layer (moe_routing) on 8 trn2 NeuronCores.

Math (per token t):
  out[t, :] = W @ x[t] + b + 2.0 * sum_n mask[n, t] * (B_n @ (A_n @ x[t]))

Strategy:
  - Data-parallel over B*T = 65536 tokens: 8192 tokens per core.
  - Streaming operands are marshaled to bf16 host-side: halves HBM traffic
    and SBUF footprint; output error ~3e-3 relative, well inside the 2e-2
    gate. PSUM accumulation stays fp32.
  - Host-side transposes give the device contiguous, partition-friendly
    layouts only:
      xt   [D_IN, TOK]   = x-shard transposed (contraction dim major)
      wt   [D_IN, D_OUT] = W.T
      at   [P, KC*NR]    = fused-A.T, pre-packed per partition
      btr  [NR, D_OUT]   = fused-B.T
      mtok [P, G, N]     = per-token routing mask, token-partition layout
  - The LoRA s = A@x projection rides the main matmul's M-tiles as a third
    small N=64 matmul per contraction chunk (2 PE cyc/token instead of 8
    for a separate [NR,SUP]-layout pass), is masked on DVE with a stride-0
    rank-broadcast AP, PE-transposed back to rank-partition layout, and
    accumulated into the base matmul's PSUM bank as a 9th contraction
    chunk. Bias is added during the PSUM->SBUF copy; output is stored
    bf16 and upcast on host.
  - Supertile 0 runs k-outer across two q-tiles so the PE consumes each
    arriving (x-chunk, W-chunk) DMA pair slower than the stream delivers;
    per-128-token output stores keep the drain tail short.
"""

import numpy as np
import ml_dtypes

import concourse.bass as bass
from concourse import bacc
from concourse.masks import make_identity
import concourse.mybir as mybir
import concourse.tile as tile
from concourse.bass_utils import run_bass_kernel_spmd

N_CORES = 8
B, T = 8, 8192
D_IN = 1024
D_OUT = 1024
N_ADAPT, R = 4, 16
NR = N_ADAPT * R  # 64
SCALING = 32.0 / 16.0

TOK = B * T // N_CORES  # 8192 tokens per core
SUP = 512               # tokens per supertile
N_SUP = TOK // SUP      # 16
SUB = 128               # tokens per matmul M-tile
N_SUB = SUP // SUB      # 4
G = N_SUP * N_SUB       # 64 M-tiles per core
P = 128
KC = D_IN // P          # 8 contraction chunks
NB = D_OUT // 512       # 2 PSUM-bank column halves

F32 = mybir.dt.float32
BF16 = mybir.dt.bfloat16
NP_BF16 = ml_dtypes.bfloat16


def build_bass(xp_bufs=4, op_bufs=4, pso_bufs=6):
    nc = bacc.Bacc(
        "TRN2", target_bir_lowering=False, debug=False, num_devices=N_CORES
    )

    xt_d = nc.dram_tensor("xt", [D_IN, TOK], BF16, kind="ExternalInput")
    wt_d = nc.dram_tensor("wt", [D_IN, D_OUT], BF16, kind="ExternalInput")
    at_d = nc.dram_tensor("at", [P, KC * NR], BF16, kind="ExternalInput")
    bt_d = nc.dram_tensor("btr", [NR, D_OUT], BF16, kind="ExternalInput")
    bias_d = nc.dram_tensor("bias", [D_OUT], BF16, kind="ExternalInput")
    mtok_d = nc.dram_tensor("mtok", [P, G * N_ADAPT], BF16, kind="ExternalInput")
    out_d = nc.dram_tensor("out", [TOK, D_OUT], BF16, kind="ExternalOutput")

    xt_r = xt_d.ap().rearrange("(kc p) t -> p kc t", p=P)
    wt_r = wt_d.ap().rearrange("(kc p) n -> p kc n", p=P)
    out_r = out_d.ap().rearrange("(s q p) n -> s q p n", q=N_SUB, p=P)
    bias_bcast = bass.AP(
        tensor=bias_d, offset=0, ap=[[0, P], [1, D_OUT]]
    )

    with tile.TileContext(nc) as tc:
        with (
            tc.tile_pool(name="const", bufs=1) as const,
            tc.tile_pool(name="xp", bufs=xp_bufs) as xp,
            tc.tile_pool(name="smqp", bufs=2) as smqp,
            tc.tile_pool(name="smtp", bufs=2) as smtp,
            tc.tile_pool(name="op", bufs=op_bufs) as op,
            tc.tile_pool(name="pso", bufs=pso_bufs, space="PSUM") as pso,
            tc.tile_pool(name="psj", bufs=1, space="PSUM") as psj,
            tc.tile_pool(name="pst", bufs=1, space="PSUM") as pst,
        ):
            w_sb = const.tile([P, KC, D_OUT], BF16)
            a_sb = const.tile([P, KC, NR], BF16)
            bt_sb = const.tile([NR, D_OUT], BF16)
            b_sb = const.tile([P, D_OUT], BF16)
            m_sb = const.tile([P, G, N_ADAPT], BF16)
            ident = const.tile([P, P], BF16)
            make_identity(nc, ident[:])
            # Preload order matters for startup latency: the first matmuls
            # need a_sb + x0 chunk 0 (sync queue) and W chunk k in order
            # (scalar queue); everything else is needed later.
            nc.gpsimd.dma_start(
                out=a_sb[:],
                in_=at_d.ap().rearrange("p (kc j) -> p kc j", kc=KC),
            )
            nc.gpsimd.dma_start(
                out=m_sb[:],
                in_=mtok_d.ap().rearrange("p (g n) -> p g n", g=G),
            )
            for k in range(KC):
                nc.scalar.dma_start(out=w_sb[:, k, :], in_=wt_r[:, k, :])
            nc.scalar.dma_start(out=bt_sb[:], in_=bt_d.ap())
            nc.scalar.dma_start(out=b_sb[:], in_=bias_bcast)

            for s in range(N_SUP):
                t0 = s * SUP
                x_sb = xp.tile([P, KC, SUP], BF16, tag="x")
                if s == 0:
                    # chunked first load: consumers of chunk k can start as
                    # soon as chunk k lands instead of after the full load
                    for k in range(0, KC, 2):
                        nc.sync.dma_start(
                            out=x_sb[:, k : k + 2, :],
                            in_=xt_r[:, k : k + 2, t0 : t0 + SUP],
                        )
                else:
                    nc.sync.dma_start(
                        out=x_sb[:], in_=xt_r[:, :, t0 : t0 + SUP]
                    )

                # per-q-tile state.  PSUM tiles are padded to a full 2KB
                # bank: accumulation-group `start` clears the whole bank, so
                # a bank must never host two in-flight groups.
                sj = {}    # [P, 64] fp32 s-projection (PSUM, bank-padded)
                smq = {}   # [P, 64] bf16 masked s, token-partition (SBUF)
                smt = {}   # [64, 128] bf16 masked s.T via PE transpose
                smt_sb = {}

                def sj_block(q):
                    # s[tok, j] += x-chunk.T @ A-chunk, rides the M-tile
                    sj[q] = psj.tile([P, 512], F32, tag="sj", name=f"sj{q}")
                    for k in range(KC):
                        nc.tensor.matmul(
                            sj[q][:, :NR],
                            x_sb[:, k, q * SUB : (q + 1) * SUB],
                            a_sb[:, k, :],
                            start=(k == 0),
                            stop=(k == KC - 1),
                        )

                def mask_q(q):
                    # rank-broadcast the per-adapter mask along r via a
                    # stride-0 AP: j = n*R + r
                    smq[q] = smqp.tile([P, NR], BF16, tag="smq", name=f"smq{q}")
                    m_bc = (
                        m_sb[:, s * N_SUB + q, :]
                        .unsqueeze(2)
                        .broadcast_to((P, N_ADAPT, R))
                    )
                    nc.vector.tensor_mul(
                        smq[q][:].rearrange("p (n r) -> p n r", n=N_ADAPT),
                        sj[q][:, :NR].rearrange("p (n r) -> p n r", n=N_ADAPT),
                        m_bc,
                    )

                def transpose_q(q):
                    smt[q] = pst.tile([NR, 1024], BF16, tag="smt", name=f"smt{q}")
                    nc.tensor.transpose(
                        smt[q][:, :SUB], smq[q][:], ident[:]
                    )

                def copy_q(q):
                    smt_sb[q] = smtp.tile(
                        [NR, SUB], BF16, tag="smtsb", name=f"smtsb{q}"
                    )
                    nc.vector.tensor_copy(smt_sb[q][:], smt[q][:, :SUB])

                def main_half(q, n, o_ps_h, skip=False):
                    ts = q * SUB
                    nsl = slice(n * 512, (n + 1) * 512)
                    for k in range(KC):
                        nc.tensor.matmul(
                            o_ps_h[:],
                            x_sb[:, k, ts : ts + SUB],
                            w_sb[:, k, nsl],
                            start=(k == 0),
                            stop=False,
                            skip_group_check=skip,
                        )

                def lora_half(q, n, o_ps_h, skip=False):
                    nsl = slice(n * 512, (n + 1) * 512)
                    nc.tensor.matmul(
                        o_ps_h[:],
                        smt_sb[q][:],
                        bt_sb[:, nsl],
                        start=False,
                        stop=True,
                        skip_group_check=skip,
                    )
                    o_sb = op.tile([P, 512], BF16, tag="o")
                    nc.vector.tensor_add(o_sb[:], o_ps_h[:], b_sb[:, nsl])
                    nc.scalar.dma_start(
                        out=out_r[s, q][:, nsl], in_=o_sb[:]
                    )

                if s == 0:
                    # Startup: k-outer across q0/q1 mains so each arriving
                    # (x-chunk, W-chunk) DMA pair unlocks PE work faster
                    # than the serialized preload stream delivers it.
                    ph01 = {}
                    for q in (0, 1, 2):
                        for n in range(NB):
                            ph01[q, n] = pso.tile(
                                [P, 512], F32, tag="ops", name=f"ops01_{q}_{n}"
                            )
                    for k in range(KC):
                        for q in (0, 1, 2):
                            for n in range(NB):
                                nsl = slice(n * 512, (n + 1) * 512)
                                nc.tensor.matmul(
                                    ph01[q, n][:],
                                    x_sb[:, k, q * SUB : (q + 1) * SUB],
                                    w_sb[:, k, nsl],
                                    start=(k == 0),
                                    stop=False,
                                    skip_group_check=True,
                                )
                    for q in range(N_SUB):
                        sj_block(q)
                        mask_q(q)
                    for q in range(N_SUB):
                        transpose_q(q)
                        copy_q(q)
                    for q in (0, 1, 2):
                        for n in range(NB):
                            lora_half(q, n, ph01[q, n], skip=True)
                    for q in (3,):
                        o_ps = {}
                        for n in range(NB):
                            o_ps[n] = pso.tile(
                                [P, 512], F32, tag="ops", name=f"ops0_{q}_{n}"
                            )
                            main_half(q, n, o_ps[n])
                        for n in range(NB):
                            lora_half(q, n, o_ps[n])
                else:
                    # Steady state: sj/transpose/copy for tile q run early,
                    # interleaved with the q-1/q main matmuls, so the LoRA-B
                    # matmul never waits on the DVE round trip.
                    o_ps = {}
                    sj_block(0)
                    mask_q(0)
                    o_ps[0, 0] = pso.tile([P, 512], F32, tag="ops", name="opsA")
                    main_half(0, 0, o_ps[0, 0])
                    sj_block(1)
                    mask_q(1)
                    transpose_q(0)
                    copy_q(0)
                    o_ps[0, 1] = pso.tile([P, 512], F32, tag="ops", name="opsB")
                    main_half(0, 1, o_ps[0, 1])
                    lora_half(0, 0, o_ps[0, 0])
                    lora_half(0, 1, o_ps[0, 1])
                    for q in (1, 2):
                        sj_block(q + 1)
                        mask_q(q + 1)
                        transpose_q(q)
                        copy_q(q)
                        for n in range(NB):
                            o_ps[q, n] = pso.tile(
                                [P, 512], F32, tag="ops", name=f"ops_{q}_{n}"
                            )
                            main_half(q, n, o_ps[q, n])
                        for n in range(NB):
                            lora_half(q, n, o_ps[q, n])
                    transpose_q(3)
                    copy_q(3)
                    for n in range(NB):
                        o_ps[3, n] = pso.tile(
                            [P, 512], F32, tag="ops", name=f"ops_3_{n}"
                        )
                        main_half(3, n, o_ps[3, n])
                    for n in range(NB):
                        lora_half(3, n, o_ps[3, n])

    nc.compile()
    return nc


_NC_CACHE = None


def _get_nc():
    global _NC_CACHE
    if _NC_CACHE is None:
        _NC_CACHE = build_bass()
    return _NC_CACHE


def make_in_maps(x, W, b, lora_A, lora_B, masks):
    x = np.ascontiguousarray(x, dtype=np.float32)
    W = np.ascontiguousarray(W, dtype=np.float32)
    b = np.ascontiguousarray(b, dtype=np.float32)
    lora_A = np.ascontiguousarray(lora_A, dtype=np.float32)
    lora_B = np.ascontiguousarray(lora_B, dtype=np.float32)
    masks = np.ascontiguousarray(masks, dtype=np.float32)

    x_flat = x.reshape(B * T, D_IN)
    A_flat = lora_A.reshape(NR, D_IN)
    B_flat = lora_B.transpose(1, 0, 2).reshape(D_OUT, NR)

    wt = np.ascontiguousarray(W.T.astype(NP_BF16))       # [D_IN, D_OUT]
    # packed [P, KC*NR]: per-partition contiguous 1KB rows (full DMA rate)
    at = np.ascontiguousarray(
        A_flat.T.astype(NP_BF16).reshape(KC, P, NR).transpose(1, 0, 2)
        .reshape(P, KC * NR)
    )
    btr = np.ascontiguousarray(B_flat.T.astype(NP_BF16))  # [NR, D_OUT]

    # per-token mask, token-partition layout [P, G*N_ADAPT]
    m_full = masks[..., 0].reshape(N_ADAPT, B * T) * np.float32(SCALING)

    in_maps = []
    for c in range(N_CORES):
        sl = slice(c * TOK, (c + 1) * TOK)
        mtok = np.ascontiguousarray(
            m_full[:, sl].T.astype(NP_BF16)             # [TOK, N]
            .reshape(G, P, N_ADAPT).transpose(1, 0, 2)  # [P, G, N]
            .reshape(P, G * N_ADAPT)
        )
        in_maps.append(
            {
                "xt": np.ascontiguousarray(x_flat[sl].astype(NP_BF16).T),
                "wt": wt,
                "at": at,
                "btr": btr,
                "bias": b.astype(NP_BF16),
                "mtok": mtok,
            }
        )
    return in_maps


def kernel(x, W, b, lora_A, lora_B, masks):
    nc = _get_nc()
    in_maps = make_in_maps(x, W, b, lora_A, lora_B, masks)
    res = run_bass_kernel_spmd(nc, in_maps, core_ids=list(range(N_CORES)))
    out = np.concatenate([r["out"] for r in res.results], axis=0)
    out = out.astype(np.float32).reshape(B, T, D_OUT)
    return out


# revision 17
# speedup vs baseline: 1.0375x; 1.0002x over previous
"""Routed-LoRA linear layer (moe_routing) on 8 trn2 NeuronCores.

Math (per token t):
  out[t, :] = W @ x[t] + b + 2.0 * sum_n mask[n, t] * (B_n @ (A_n @ x[t]))

Strategy:
  - Data-parallel over B*T = 65536 tokens: 8192 tokens per core.
  - Streaming operands are marshaled to bf16 host-side: halves HBM traffic
    and SBUF footprint; output error ~3e-3 relative, well inside the 2e-2
    gate. PSUM accumulation stays fp32.
  - Host-side transposes give the device contiguous, partition-friendly
    layouts only:
      xt   [D_IN, TOK]   = x-shard transposed (contraction dim major)
      wt   [D_IN, D_OUT] = W.T
      at   [P, KC*NR]    = fused-A.T, pre-packed per partition
      btr  [NR, D_OUT]   = fused-B.T
      mtok [P, G, N]     = per-token routing mask, token-partition layout
  - The LoRA s = A@x projection rides the main matmul's M-tiles as a third
    small N=64 matmul per contraction chunk (2 PE cyc/token instead of 8
    for a separate [NR,SUP]-layout pass), is masked on DVE with a stride-0
    rank-broadcast AP, PE-transposed back to rank-partition layout, and
    accumulated into the base matmul's PSUM bank as a 9th contraction
    chunk. Bias is added during the PSUM->SBUF copy; output is stored
    bf16 and upcast on host.
  - Supertile 0 runs k-outer across two q-tiles so the PE consumes each
    arriving (x-chunk, W-chunk) DMA pair slower than the stream delivers;
    per-128-token output stores keep the drain tail short.
"""

import numpy as np
import ml_dtypes

import concourse.bass as bass
from concourse import bacc
from concourse.masks import make_identity
import concourse.mybir as mybir
import concourse.tile as tile
from concourse.bass_utils import run_bass_kernel_spmd

N_CORES = 8
B, T = 8, 8192
D_IN = 1024
D_OUT = 1024
N_ADAPT, R = 4, 16
NR = N_ADAPT * R  # 64
SCALING = 32.0 / 16.0

TOK = B * T // N_CORES  # 8192 tokens per core
SUP = 512               # tokens per supertile
N_SUP = TOK // SUP      # 16
SUB = 128               # tokens per matmul M-tile
N_SUB = SUP // SUB      # 4
G = N_SUP * N_SUB       # 64 M-tiles per core
P = 128
KC = D_IN // P          # 8 contraction chunks
NB = D_OUT // 512       # 2 PSUM-bank column halves

F32 = mybir.dt.float32
BF16 = mybir.dt.bfloat16
NP_BF16 = ml_dtypes.bfloat16


def build_bass(xp_bufs=4, op_bufs=4, pso_bufs=6):
    nc = bacc.Bacc(
        "TRN2", target_bir_lowering=False, debug=False, num_devices=N_CORES
    )

    xt_d = nc.dram_tensor("xt", [D_IN, TOK], BF16, kind="ExternalInput")
    wt_d = nc.dram_tensor("wt", [D_IN, D_OUT], BF16, kind="ExternalInput")
    at_d = nc.dram_tensor("at", [P, KC * NR], BF16, kind="ExternalInput")
    bt_d = nc.dram_tensor("btr", [NR, D_OUT], BF16, kind="ExternalInput")
    bias_d = nc.dram_tensor("bias", [D_OUT], BF16, kind="ExternalInput")
    mtok_d = nc.dram_tensor("mtok", [P, G * N_ADAPT], BF16, kind="ExternalInput")
    out_d = nc.dram_tensor("out", [TOK, D_OUT], BF16, kind="ExternalOutput")

    xt_r = xt_d.ap().rearrange("(kc p) t -> p kc t", p=P)
    wt_r = wt_d.ap().rearrange("(kc p) n -> p kc n", p=P)
    out_r = out_d.ap().rearrange("(s q p) n -> s q p n", q=N_SUB, p=P)
    bias_bcast = bass.AP(
        tensor=bias_d, offset=0, ap=[[0, P], [1, D_OUT]]
    )

    with tile.TileContext(nc) as tc:
        with (
            tc.tile_pool(name="const", bufs=1) as const,
            tc.tile_pool(name="xp", bufs=xp_bufs) as xp,
            tc.tile_pool(name="smqp", bufs=2) as smqp,
            tc.tile_pool(name="smtp", bufs=2) as smtp,
            tc.tile_pool(name="op", bufs=op_bufs) as op,
            tc.tile_pool(name="pso", bufs=pso_bufs, space="PSUM") as pso,
            tc.tile_pool(name="psj", bufs=1, space="PSUM") as psj,
            tc.tile_pool(name="pst", bufs=1, space="PSUM") as pst,
        ):
            w_sb = const.tile([P, KC, D_OUT], BF16)
            a_sb = const.tile([P, KC, NR], BF16)
            bt_sb = const.tile([NR, D_OUT], BF16)
            b_sb = const.tile([P, D_OUT], BF16)
            m_sb = const.tile([P, G, N_ADAPT], BF16)
            ident = const.tile([P, P], BF16)
            make_identity(nc, ident[:])
            # Preload order matters for startup latency: the first matmuls
            # need a_sb + x0 chunk 0 (sync queue) and W chunk k in order
            # (scalar queue); everything else is needed later.
            for k in range(KC):
                nc.scalar.dma_start(out=w_sb[:, k, :], in_=wt_r[:, k, :])
            nc.scalar.dma_start(out=bt_sb[:], in_=bt_d.ap())
            nc.scalar.dma_start(out=b_sb[:], in_=bias_bcast)
            # a/m ride the back of the scalar preload queue: in place well
            # before the post-k-outer sj/mask phase needs them (~16us), and
            # never ahead of x0/W in the serialized DMA stream
            nc.scalar.dma_start(
                out=a_sb[:],
                in_=at_d.ap().rearrange("p (kc j) -> p kc j", kc=KC),
            )
            nc.scalar.dma_start(
                out=m_sb[:],
                in_=mtok_d.ap().rearrange("p (g n) -> p g n", g=G),
            )

            for s in range(N_SUP):
                t0 = s * SUP
                x_sb = xp.tile([P, KC, SUP], BF16, tag="x")
                if s == 0:
                    # chunked first load: consumers of chunk k can start as
                    # soon as chunk k lands instead of after the full load
                    for k in range(0, KC, 2):
                        nc.sync.dma_start(
                            out=x_sb[:, k : k + 2, :],
                            in_=xt_r[:, k : k + 2, t0 : t0 + SUP],
                        )
                else:
                    nc.sync.dma_start(
                        out=x_sb[:], in_=xt_r[:, :, t0 : t0 + SUP]
                    )

                # per-q-tile state.  PSUM tiles are padded to a full 2KB
                # bank: accumulation-group `start` clears the whole bank, so
                # a bank must never host two in-flight groups.
                sj = {}    # [P, 64] fp32 s-projection (PSUM, bank-padded)
                smq = {}   # [P, 64] bf16 masked s, token-partition (SBUF)
                smt = {}   # [64, 128] bf16 masked s.T via PE transpose
                smt_sb = {}

                def sj_block(q):
                    # s[tok, j] += x-chunk.T @ A-chunk, rides the M-tile
                    sj[q] = psj.tile([P, 512], F32, tag="sj", name=f"sj{q}")
                    for k in range(KC):
                        nc.tensor.matmul(
                            sj[q][:, :NR],
                            x_sb[:, k, q * SUB : (q + 1) * SUB],
                            a_sb[:, k, :],
                            start=(k == 0),
                            stop=(k == KC - 1),
                        )

                def mask_q(q):
                    # rank-broadcast the per-adapter mask along r via a
                    # stride-0 AP: j = n*R + r
                    smq[q] = smqp.tile([P, NR], BF16, tag="smq", name=f"smq{q}")
                    m_bc = (
                        m_sb[:, s * N_SUB + q, :]
                        .unsqueeze(2)
                        .broadcast_to((P, N_ADAPT, R))
                    )
                    nc.vector.tensor_mul(
                        smq[q][:].rearrange("p (n r) -> p n r", n=N_ADAPT),
                        sj[q][:, :NR].rearrange("p (n r) -> p n r", n=N_ADAPT),
                        m_bc,
                    )

                def transpose_q(q):
                    smt[q] = pst.tile([NR, 1024], BF16, tag="smt", name=f"smt{q}")
                    nc.tensor.transpose(
                        smt[q][:, :SUB], smq[q][:], ident[:]
                    )

                def copy_q(q):
                    smt_sb[q] = smtp.tile(
                        [NR, SUB], BF16, tag="smtsb", name=f"smtsb{q}"
                    )
                    nc.vector.tensor_copy(smt_sb[q][:], smt[q][:, :SUB])

                def main_half(q, n, o_ps_h, skip=False):
                    ts = q * SUB
                    nsl = slice(n * 512, (n + 1) * 512)
                    for k in range(KC):
                        nc.tensor.matmul(
                            o_ps_h[:],
                            x_sb[:, k, ts : ts + SUB],
                            w_sb[:, k, nsl],
                            start=(k == 0),
                            stop=False,
                            skip_group_check=skip,
                        )

                def lora_half(q, n, o_ps_h, skip=False):
                    nsl = slice(n * 512, (n + 1) * 512)
                    nc.tensor.matmul(
                        o_ps_h[:],
                        smt_sb[q][:],
                        bt_sb[:, nsl],
                        start=False,
                        stop=True,
                        skip_group_check=skip,
                    )
                    o_sb = op.tile([P, 512], BF16, tag="o")
                    nc.vector.tensor_add(o_sb[:], o_ps_h[:], b_sb[:, nsl])
                    nc.scalar.dma_start(
                        out=out_r[s, q][:, nsl], in_=o_sb[:]
                    )

                if s == 0:
                    # Startup: k-outer across q0/q1 mains so each arriving
                    # (x-chunk, W-chunk) DMA pair unlocks PE work faster
                    # than the serialized preload stream delivers it.
                    ph01 = {}
                    for q in (0, 1, 2):
                        for n in range(NB):
                            ph01[q, n] = pso.tile(
                                [P, 512], F32, tag="ops", name=f"ops01_{q}_{n}"
                            )
                    for k in range(KC):
                        for q in (0, 1, 2):
                            for n in range(NB):
                                nsl = slice(n * 512, (n + 1) * 512)
                                nc.tensor.matmul(
                                    ph01[q, n][:],
                                    x_sb[:, k, q * SUB : (q + 1) * SUB],
                                    w_sb[:, k, nsl],
                                    start=(k == 0),
                                    stop=False,
                                    skip_group_check=True,
                                )
                    for q in range(N_SUB):
                        sj_block(q)
                        mask_q(q)
                    for q in range(N_SUB):
                        transpose_q(q)
                        copy_q(q)
                    for q in (0, 1, 2):
                        for n in range(NB):
                            lora_half(q, n, ph01[q, n], skip=True)
                    for q in (3,):
                        o_ps = {}
                        for n in range(NB):
                            o_ps[n] = pso.tile(
                                [P, 512], F32, tag="ops", name=f"ops0_{q}_{n}"
                            )
                            main_half(q, n, o_ps[n])
                        for n in range(NB):
                            lora_half(q, n, o_ps[n])
                else:
                    # Steady state: sj/transpose/copy for tile q run early,
                    # interleaved with the q-1/q main matmuls, so the LoRA-B
                    # matmul never waits on the DVE round trip.
                    o_ps = {}
                    sj_block(0)
                    mask_q(0)
                    o_ps[0, 0] = pso.tile([P, 512], F32, tag="ops", name="opsA")
                    main_half(0, 0, o_ps[0, 0])
                    sj_block(1)
                    mask_q(1)
                    transpose_q(0)
                    copy_q(0)
                    o_ps[0, 1] = pso.tile([P, 512], F32, tag="ops", name="opsB")
                    main_half(0, 1, o_ps[0, 1])
                    lora_half(0, 0, o_ps[0, 0])
                    lora_half(0, 1, o_ps[0, 1])
                    for q in (1, 2):
                        sj_block(q + 1)
                        mask_q(q + 1)
                        transpose_q(q)
                        copy_q(q)
                        for n in range(NB):
                            o_ps[q, n] = pso.tile(
                                [P, 512], F32, tag="ops", name=f"ops_{q}_{n}"
                            )
                            main_half(q, n, o_ps[q, n])
                        for n in range(NB):
                            lora_half(q, n, o_ps[q, n])
                    transpose_q(3)
                    copy_q(3)
                    for n in range(NB):
                        o_ps[3, n] = pso.tile(
                            [P, 512], F32, tag="ops", name=f"ops_3_{n}"
                        )
                        main_half(3, n, o_ps[3, n])
                    for n in range(NB):
                        lora_half(3, n, o_ps[3, n])

    nc.compile()
    return nc


_NC_CACHE = None


def _get_nc():
    global _NC_CACHE
    if _NC_CACHE is None:
        _NC_CACHE = build_bass()
    return _NC_CACHE


def make_in_maps(x, W, b, lora_A, lora_B, masks):
    x = np.ascontiguousarray(x, dtype=np.float32)
    W = np.ascontiguousarray(W, dtype=np.float32)
    b = np.ascontiguousarray(b, dtype=np.float32)
    lora_A = np.ascontiguousarray(lora_A, dtype=np.float32)
    lora_B = np.ascontiguousarray(lora_B, dtype=np.float32)
    masks = np.ascontiguousarray(masks, dtype=np.float32)

    x_flat = x.reshape(B * T, D_IN)
    A_flat = lora_A.reshape(NR, D_IN)
    B_flat = lora_B.transpose(1, 0, 2).reshape(D_OUT, NR)

    wt = np.ascontiguousarray(W.T.astype(NP_BF16))       # [D_IN, D_OUT]
    # packed [P, KC*NR]: per-partition contiguous 1KB rows (full DMA rate)
    at = np.ascontiguousarray(
        A_flat.T.astype(NP_BF16).reshape(KC, P, NR).transpose(1, 0, 2)
        .reshape(P, KC * NR)
    )
    btr = np.ascontiguousarray(B_flat.T.astype(NP_BF16))  # [NR, D_OUT]

    # per-token mask, token-partition layout [P, G*N_ADAPT]
    m_full = masks[..., 0].reshape(N_ADAPT, B * T) * np.float32(SCALING)

    in_maps = []
    for c in range(N_CORES):
        sl = slice(c * TOK, (c + 1) * TOK)
        mtok = np.ascontiguousarray(
            m_full[:, sl].T.astype(NP_BF16)             # [TOK, N]
            .reshape(G, P, N_ADAPT).transpose(1, 0, 2)  # [P, G, N]
            .reshape(P, G * N_ADAPT)
        )
        in_maps.append(
            {
                "xt": np.ascontiguousarray(x_flat[sl].astype(NP_BF16).T),
                "wt": wt,
                "at": at,
                "btr": btr,
                "bias": b.astype(NP_BF16),
                "mtok": mtok,
            }
        )
    return in_maps


def kernel(x, W, b, lora_A, lora_B, masks):
    nc = _get_nc()
    in_maps = make_in_maps(x, W, b, lora_A, lora_B, masks)
    res = run_bass_kernel_spmd(nc, in_maps, core_ids=list(range(N_CORES)))
    out = np.concatenate([r["out"] for r in res.results], axis=0)
    out = out.astype(np.float32).reshape(B, T, D_OUT)
    return out


# revision 18
# speedup vs baseline: 1.0399x; 1.0023x over previous
"""Routed-LoRA linear layer (moe_routing) on 8 trn2 NeuronCores.

Math (per token t):
  out[t, :] = W @ x[t] + b + 2.0 * sum_n mask[n, t] * (B_n @ (A_n @ x[t]))

Strategy:
  - Data-parallel over B*T = 65536 tokens: 8192 tokens per core.
  - Streaming operands are marshaled to bf16 host-side: halves HBM traffic
    and SBUF footprint; output error ~3e-3 relative, well inside the 2e-2
    gate. PSUM accumulation stays fp32.
  - Host-side transposes give the device contiguous, partition-friendly
    layouts only:
      xt   [D_IN, TOK]   = x-shard transposed (contraction dim major)
      wt   [D_IN, D_OUT] = W.T
      at   [P, KC*NR]    = fused-A.T, pre-packed per partition
      btr  [NR, D_OUT]   = fused-B.T
      mtok [P, G, N]     = per-token routing mask, token-partition layout
  - The LoRA s = A@x projection rides the main matmul's M-tiles as a third
    small N=64 matmul per contraction chunk (2 PE cyc/token instead of 8
    for a separate [NR,SUP]-layout pass), is masked on DVE with a stride-0
    rank-broadcast AP, PE-transposed back to rank-partition layout, and
    accumulated into the base matmul's PSUM bank as a 9th contraction
    chunk. Bias is added during the PSUM->SBUF copy; output is stored
    bf16 and upcast on host.
  - Supertile 0 runs k-outer across two q-tiles so the PE consumes each
    arriving (x-chunk, W-chunk) DMA pair slower than the stream delivers;
    per-128-token output stores keep the drain tail short.
"""

import numpy as np
import ml_dtypes

import concourse.bass as bass
from concourse import bacc
from concourse.masks import make_identity
import concourse.mybir as mybir
import concourse.tile as tile
from concourse.bass_utils import run_bass_kernel_spmd

N_CORES = 8
B, T = 8, 8192
D_IN = 1024
D_OUT = 1024
N_ADAPT, R = 4, 16
NR = N_ADAPT * R  # 64
SCALING = 32.0 / 16.0

TOK = B * T // N_CORES  # 8192 tokens per core
SUP = 512               # tokens per supertile
N_SUP = TOK // SUP      # 16
SUB = 128               # tokens per matmul M-tile
N_SUB = SUP // SUB      # 4
G = N_SUP * N_SUB       # 64 M-tiles per core
P = 128
KC = D_IN // P          # 8 contraction chunks
NB = D_OUT // 512       # 2 PSUM-bank column halves

F32 = mybir.dt.float32
BF16 = mybir.dt.bfloat16
NP_BF16 = ml_dtypes.bfloat16


def build_bass(xp_bufs=4, op_bufs=4, pso_bufs=6):
    nc = bacc.Bacc(
        "TRN2", target_bir_lowering=False, debug=False, num_devices=N_CORES
    )

    xt_d = nc.dram_tensor("xt", [D_IN, TOK], BF16, kind="ExternalInput")
    wt_d = nc.dram_tensor("wt", [D_IN, D_OUT], BF16, kind="ExternalInput")
    at_d = nc.dram_tensor("at", [P, KC * NR], BF16, kind="ExternalInput")
    bt_d = nc.dram_tensor("btr", [NR, D_OUT], BF16, kind="ExternalInput")
    bias_d = nc.dram_tensor("bias", [D_OUT], BF16, kind="ExternalInput")
    mtok_d = nc.dram_tensor("mtok", [P, G * N_ADAPT], BF16, kind="ExternalInput")
    out_d = nc.dram_tensor("out", [TOK, D_OUT], BF16, kind="ExternalOutput")

    xt_r = xt_d.ap().rearrange("(kc p) t -> p kc t", p=P)
    wt_r = wt_d.ap().rearrange("(kc p) n -> p kc n", p=P)
    out_r = out_d.ap().rearrange("(s q p) n -> s q p n", q=N_SUB, p=P)
    bias_bcast = bass.AP(
        tensor=bias_d, offset=0, ap=[[0, P], [1, D_OUT]]
    )

    with tile.TileContext(nc) as tc:
        with (
            tc.tile_pool(name="const", bufs=1) as const,
            tc.tile_pool(name="xp", bufs=xp_bufs) as xp,
            tc.tile_pool(name="smqp", bufs=2) as smqp,
            tc.tile_pool(name="smtp", bufs=2) as smtp,
            tc.tile_pool(name="op", bufs=op_bufs) as op,
            tc.tile_pool(name="pso", bufs=pso_bufs, space="PSUM") as pso,
            tc.tile_pool(name="psj", bufs=1, space="PSUM") as psj,
            tc.tile_pool(name="pst", bufs=1, space="PSUM") as pst,
        ):
            w_sb = const.tile([P, KC, D_OUT], BF16)
            a_sb = const.tile([P, KC, NR], BF16)
            bt_sb = const.tile([NR, D_OUT], BF16)
            b_sb = const.tile([P, D_OUT], BF16)
            m_sb = const.tile([P, G, N_ADAPT], BF16)
            ident = const.tile([P, P], BF16)
            make_identity(nc, ident[:])
            # Preload order matters for startup latency: the first matmuls
            # need a_sb + x0 chunk 0 (sync queue) and W chunk k in order
            # (scalar queue); everything else is needed later.
            for k in range(KC):
                nc.scalar.dma_start(out=w_sb[:, k, :], in_=wt_r[:, k, :])
            nc.scalar.dma_start(out=bt_sb[:], in_=bt_d.ap())
            nc.scalar.dma_start(out=b_sb[:], in_=bias_bcast)
            # a/m ride the back of the scalar preload queue: in place well
            # before the post-k-outer sj/mask phase needs them (~16us), and
            # never ahead of x0/W in the serialized DMA stream
            nc.scalar.dma_start(
                out=a_sb[:],
                in_=at_d.ap().rearrange("p (kc j) -> p kc j", kc=KC),
            )
            nc.scalar.dma_start(
                out=m_sb[:],
                in_=mtok_d.ap().rearrange("p (g n) -> p g n", g=G),
            )

            for s in range(N_SUP):
                t0 = s * SUP
                x_sb = xp.tile([P, KC, SUP], BF16, tag="x")
                if s == 0:
                    # chunked first load: consumers of chunk k can start as
                    # soon as chunk k lands instead of after the full load
                    for k in (0, 1):
                        nc.sync.dma_start(
                            out=x_sb[:, k, :], in_=xt_r[:, k, t0 : t0 + SUP]
                        )
                    for k in range(2, KC, 2):
                        nc.sync.dma_start(
                            out=x_sb[:, k : k + 2, :],
                            in_=xt_r[:, k : k + 2, t0 : t0 + SUP],
                        )
                else:
                    nc.sync.dma_start(
                        out=x_sb[:], in_=xt_r[:, :, t0 : t0 + SUP]
                    )

                # per-q-tile state.  PSUM tiles are padded to a full 2KB
                # bank: accumulation-group `start` clears the whole bank, so
                # a bank must never host two in-flight groups.
                sj = {}    # [P, 64] fp32 s-projection (PSUM, bank-padded)
                smq = {}   # [P, 64] bf16 masked s, token-partition (SBUF)
                smt = {}   # [64, 128] bf16 masked s.T via PE transpose
                smt_sb = {}

                def sj_block(q):
                    # s[tok, j] += x-chunk.T @ A-chunk, rides the M-tile
                    sj[q] = psj.tile([P, 512], F32, tag="sj", name=f"sj{q}")
                    for k in range(KC):
                        nc.tensor.matmul(
                            sj[q][:, :NR],
                            x_sb[:, k, q * SUB : (q + 1) * SUB],
                            a_sb[:, k, :],
                            start=(k == 0),
                            stop=(k == KC - 1),
                        )

                def mask_q(q):
                    # rank-broadcast the per-adapter mask along r via a
                    # stride-0 AP: j = n*R + r
                    smq[q] = smqp.tile([P, NR], BF16, tag="smq", name=f"smq{q}")
                    m_bc = (
                        m_sb[:, s * N_SUB + q, :]
                        .unsqueeze(2)
                        .broadcast_to((P, N_ADAPT, R))
                    )
                    nc.vector.tensor_mul(
                        smq[q][:].rearrange("p (n r) -> p n r", n=N_ADAPT),
                        sj[q][:, :NR].rearrange("p (n r) -> p n r", n=N_ADAPT),
                        m_bc,
                    )

                def transpose_q(q):
                    smt[q] = pst.tile([NR, 1024], BF16, tag="smt", name=f"smt{q}")
                    nc.tensor.transpose(
                        smt[q][:, :SUB], smq[q][:], ident[:]
                    )

                def copy_q(q):
                    smt_sb[q] = smtp.tile(
                        [NR, SUB], BF16, tag="smtsb", name=f"smtsb{q}"
                    )
                    nc.vector.tensor_copy(smt_sb[q][:], smt[q][:, :SUB])

                def main_half(q, n, o_ps_h, skip=False):
                    ts = q * SUB
                    nsl = slice(n * 512, (n + 1) * 512)
                    for k in range(KC):
                        nc.tensor.matmul(
                            o_ps_h[:],
                            x_sb[:, k, ts : ts + SUB],
                            w_sb[:, k, nsl],
                            start=(k == 0),
                            stop=False,
                            skip_group_check=skip,
                        )

                def lora_half(q, n, o_ps_h, skip=False):
                    nsl = slice(n * 512, (n + 1) * 512)
                    nc.tensor.matmul(
                        o_ps_h[:],
                        smt_sb[q][:],
                        bt_sb[:, nsl],
                        start=False,
                        stop=True,
                        skip_group_check=skip,
                    )
                    o_sb = op.tile([P, 512], BF16, tag="o")
                    nc.vector.tensor_add(o_sb[:], o_ps_h[:], b_sb[:, nsl])
                    # the closing store rides the idle sync queue so its
                    # descriptor generation overlaps the scalar queue's
                    eng = nc.sync if (s == N_SUP - 1 and q == N_SUB - 1
                                      and n == NB - 1) else nc.scalar
                    eng.dma_start(out=out_r[s, q][:, nsl], in_=o_sb[:])

                if s == 0:
                    # Startup: k-outer across q0/q1 mains so each arriving
                    # (x-chunk, W-chunk) DMA pair unlocks PE work faster
                    # than the serialized preload stream delivers it.
                    ph01 = {}
                    for q in (0, 1, 2):
                        for n in range(NB):
                            ph01[q, n] = pso.tile(
                                [P, 512], F32, tag="ops", name=f"ops01_{q}_{n}"
                            )
                    for k in range(KC):
                        for q in (0, 1, 2):
                            for n in range(NB):
                                nsl = slice(n * 512, (n + 1) * 512)
                                nc.tensor.matmul(
                                    ph01[q, n][:],
                                    x_sb[:, k, q * SUB : (q + 1) * SUB],
                                    w_sb[:, k, nsl],
                                    start=(k == 0),
                                    stop=False,
                                    skip_group_check=True,
                                )
                    for q in range(N_SUB):
                        sj_block(q)
                        mask_q(q)
                    for q in range(N_SUB):
                        transpose_q(q)
                        copy_q(q)
                    for q in (0, 1, 2):
                        for n in range(NB):
                            lora_half(q, n, ph01[q, n], skip=True)
                    for q in (3,):
                        o_ps = {}
                        for n in range(NB):
                            o_ps[n] = pso.tile(
                                [P, 512], F32, tag="ops", name=f"ops0_{q}_{n}"
                            )
                            main_half(q, n, o_ps[n])
                        for n in range(NB):
                            lora_half(q, n, o_ps[n])
                else:
                    # Steady state: sj/transpose/copy for tile q run early,
                    # interleaved with the q-1/q main matmuls, so the LoRA-B
                    # matmul never waits on the DVE round trip.
                    o_ps = {}
                    sj_block(0)
                    mask_q(0)
                    o_ps[0, 0] = pso.tile([P, 512], F32, tag="ops", name="opsA")
                    main_half(0, 0, o_ps[0, 0])
                    sj_block(1)
                    mask_q(1)
                    transpose_q(0)
                    copy_q(0)
                    o_ps[0, 1] = pso.tile([P, 512], F32, tag="ops", name="opsB")
                    main_half(0, 1, o_ps[0, 1])
                    lora_half(0, 0, o_ps[0, 0])
                    lora_half(0, 1, o_ps[0, 1])
                    for q in (1, 2):
                        sj_block(q + 1)
                        mask_q(q + 1)
                        transpose_q(q)
                        copy_q(q)
                        for n in range(NB):
                            o_ps[q, n] = pso.tile(
                                [P, 512], F32, tag="ops", name=f"ops_{q}_{n}"
                            )
                            main_half(q, n, o_ps[q, n])
                        for n in range(NB):
                            lora_half(q, n, o_ps[q, n])
                    transpose_q(3)
                    copy_q(3)
                    for n in range(NB):
                        o_ps[3, n] = pso.tile(
                            [P, 512], F32, tag="ops", name=f"ops_3_{n}"
                        )
                        main_half(3, n, o_ps[3, n])
                    for n in range(NB):
                        lora_half(3, n, o_ps[3, n])

    nc.compile()
    return nc


_NC_CACHE = None


def _get_nc():
    global _NC_CACHE
    if _NC_CACHE is None:
        _NC_CACHE = build_bass()
    return _NC_CACHE


def make_in_maps(x, W, b, lora_A, lora_B, masks):
    x = np.ascontiguousarray(x, dtype=np.float32)
    W = np.ascontiguousarray(W, dtype=np.float32)
    b = np.ascontiguousarray(b, dtype=np.float32)
    lora_A = np.ascontiguousarray(lora_A, dtype=np.float32)
    lora_B = np.ascontiguousarray(lora_B, dtype=np.float32)
    masks = np.ascontiguousarray(masks, dtype=np.float32)

    x_flat = x.reshape(B * T, D_IN)
    A_flat = lora_A.reshape(NR, D_IN)
    B_flat = lora_B.transpose(1, 0, 2).reshape(D_OUT, NR)

    wt = np.ascontiguousarray(W.T.astype(NP_BF16))       # [D_IN, D_OUT]
    # packed [P, KC*NR]: per-partition contiguous 1KB rows (full DMA rate)
    at = np.ascontiguousarray(
        A_flat.T.astype(NP_BF16).reshape(KC, P, NR).transpose(1, 0, 2)
        .reshape(P, KC * NR)
    )
    btr = np.ascontiguousarray(B_flat.T.astype(NP_BF16))  # [NR, D_OUT]

    # per-token mask, token-partition layout [P, G*N_ADAPT]
    m_full = masks[..., 0].reshape(N_ADAPT, B * T) * np.float32(SCALING)

    in_maps = []
    for c in range(N_CORES):
        sl = slice(c * TOK, (c + 1) * TOK)
        mtok = np.ascontiguousarray(
            m_full[:, sl].T.astype(NP_BF16)             # [TOK, N]
            .reshape(G, P, N_ADAPT).transpose(1, 0, 2)  # [P, G, N]
            .reshape(P, G * N_ADAPT)
        )
        in_maps.append(
            {
                "xt": np.ascontiguousarray(x_flat[sl].astype(NP_BF16).T),
                "wt": wt,
                "at": at,
                "btr": btr,
                "bias": b.astype(NP_BF16),
                "mtok": mtok,
            }
        )
    return in_maps


def kernel(x, W, b, lora_A, lora_B, masks):
    nc = _get_nc()
    in_maps = make_in_maps(x, W, b, lora_A, lora_B, masks)
    res = run_bass_kernel_spmd(nc, in_maps, core_ids=list(range(N_CORES)))
    out = np.concatenate([r["out"] for r in res.results], axis=0)
    out = out.astype(np.float32).reshape(B, T, D_OUT)
    return out


# revision 19
# speedup vs baseline: 1.0406x; 1.0006x over previous
"""Routed-LoRA linear layer (moe_routing) on 8 trn2 NeuronCores.

Math (per token t):
  out[t, :] = W @ x[t] + b + 2.0 * sum_n mask[n, t] * (B_n @ (A_n @ x[t]))

Strategy:
  - Data-parallel over B*T = 65536 tokens: 8192 tokens per core.
  - Streaming operands are marshaled to bf16 host-side: halves HBM traffic
    and SBUF footprint; output error ~3e-3 relative, well inside the 2e-2
    gate. PSUM accumulation stays fp32.
  - Host-side transposes give the device contiguous, partition-friendly
    layouts only:
      xt   [D_IN, TOK]   = x-shard transposed (contraction dim major)
      wt   [D_IN, D_OUT] = W.T
      at   [P, KC*NR]    = fused-A.T, pre-packed per partition
      btr  [NR, D_OUT]   = fused-B.T
      mtok [P, G, N]     = per-token routing mask, token-partition layout
  - The LoRA s = A@x projection rides the main matmul's M-tiles as a third
    small N=64 matmul per contraction chunk (2 PE cyc/token instead of 8
    for a separate [NR,SUP]-layout pass), is masked on DVE with a stride-0
    rank-broadcast AP, PE-transposed back to rank-partition layout, and
    accumulated into the base matmul's PSUM bank as a 9th contraction
    chunk. Bias is added during the PSUM->SBUF copy; output is stored
    bf16 and upcast on host.
  - Supertile 0 runs k-outer across two q-tiles so the PE consumes each
    arriving (x-chunk, W-chunk) DMA pair slower than the stream delivers;
    per-128-token output stores keep the drain tail short.
"""

import numpy as np
import ml_dtypes

import concourse.bass as bass
from concourse import bacc
from concourse.masks import make_identity
import concourse.mybir as mybir
import concourse.tile as tile
from concourse.bass_utils import run_bass_kernel_spmd

N_CORES = 8
B, T = 8, 8192
D_IN = 1024
D_OUT = 1024
N_ADAPT, R = 4, 16
NR = N_ADAPT * R  # 64
SCALING = 32.0 / 16.0

TOK = B * T // N_CORES  # 8192 tokens per core
SUP = 512               # tokens per supertile
N_SUP = TOK // SUP      # 16
SUB = 128               # tokens per matmul M-tile
N_SUB = SUP // SUB      # 4
G = N_SUP * N_SUB       # 64 M-tiles per core
P = 128
KC = D_IN // P          # 8 contraction chunks
NB = D_OUT // 512       # 2 PSUM-bank column halves

F32 = mybir.dt.float32
BF16 = mybir.dt.bfloat16
NP_BF16 = ml_dtypes.bfloat16


def build_bass(xp_bufs=4, op_bufs=6, pso_bufs=6):
    nc = bacc.Bacc(
        "TRN2", target_bir_lowering=False, debug=False, num_devices=N_CORES
    )

    xt_d = nc.dram_tensor("xt", [D_IN, TOK], BF16, kind="ExternalInput")
    wt_d = nc.dram_tensor("wt", [D_IN, D_OUT], BF16, kind="ExternalInput")
    at_d = nc.dram_tensor("at", [P, KC * NR], BF16, kind="ExternalInput")
    bt_d = nc.dram_tensor("btr", [NR, D_OUT], BF16, kind="ExternalInput")
    bias_d = nc.dram_tensor("bias", [D_OUT], BF16, kind="ExternalInput")
    mtok_d = nc.dram_tensor("mtok", [P, G * N_ADAPT], BF16, kind="ExternalInput")
    out_d = nc.dram_tensor("out", [TOK, D_OUT], BF16, kind="ExternalOutput")

    xt_r = xt_d.ap().rearrange("(kc p) t -> p kc t", p=P)
    wt_r = wt_d.ap().rearrange("(kc p) n -> p kc n", p=P)
    out_r = out_d.ap().rearrange("(s q p) n -> s q p n", q=N_SUB, p=P)
    bias_bcast = bass.AP(
        tensor=bias_d, offset=0, ap=[[0, P], [1, D_OUT]]
    )

    with tile.TileContext(nc) as tc:
        with (
            tc.tile_pool(name="const", bufs=1) as const,
            tc.tile_pool(name="xp", bufs=xp_bufs) as xp,
            tc.tile_pool(name="smqp", bufs=2) as smqp,
            tc.tile_pool(name="smtp", bufs=2) as smtp,
            tc.tile_pool(name="op", bufs=op_bufs) as op,
            tc.tile_pool(name="pso", bufs=pso_bufs, space="PSUM") as pso,
            tc.tile_pool(name="psj", bufs=1, space="PSUM") as psj,
            tc.tile_pool(name="pst", bufs=1, space="PSUM") as pst,
        ):
            w_sb = const.tile([P, KC, D_OUT], BF16)
            a_sb = const.tile([P, KC, NR], BF16)
            bt_sb = const.tile([NR, D_OUT], BF16)
            b_sb = const.tile([P, D_OUT], BF16)
            m_sb = const.tile([P, G, N_ADAPT], BF16)
            ident = const.tile([P, P], BF16)
            make_identity(nc, ident[:])
            # Preload order matters for startup latency: the first matmuls
            # need a_sb + x0 chunk 0 (sync queue) and W chunk k in order
            # (scalar queue); everything else is needed later.
            for k in range(KC):
                nc.scalar.dma_start(out=w_sb[:, k, :], in_=wt_r[:, k, :])
            nc.scalar.dma_start(out=bt_sb[:], in_=bt_d.ap())
            nc.scalar.dma_start(out=b_sb[:], in_=bias_bcast)
            # a/m ride the back of the scalar preload queue: in place well
            # before the post-k-outer sj/mask phase needs them (~16us), and
            # never ahead of x0/W in the serialized DMA stream
            nc.scalar.dma_start(
                out=a_sb[:],
                in_=at_d.ap().rearrange("p (kc j) -> p kc j", kc=KC),
            )
            nc.scalar.dma_start(
                out=m_sb[:],
                in_=mtok_d.ap().rearrange("p (g n) -> p g n", g=G),
            )

            for s in range(N_SUP):
                t0 = s * SUP
                x_sb = xp.tile([P, KC, SUP], BF16, tag="x")
                if s == 0:
                    # chunked first load: consumers of chunk k can start as
                    # soon as chunk k lands instead of after the full load
                    for k in (0, 1):
                        nc.sync.dma_start(
                            out=x_sb[:, k, :], in_=xt_r[:, k, t0 : t0 + SUP]
                        )
                    for k in range(2, KC, 2):
                        nc.sync.dma_start(
                            out=x_sb[:, k : k + 2, :],
                            in_=xt_r[:, k : k + 2, t0 : t0 + SUP],
                        )
                else:
                    nc.sync.dma_start(
                        out=x_sb[:], in_=xt_r[:, :, t0 : t0 + SUP]
                    )

                # per-q-tile state.  PSUM tiles are padded to a full 2KB
                # bank: accumulation-group `start` clears the whole bank, so
                # a bank must never host two in-flight groups.
                sj = {}    # [P, 64] fp32 s-projection (PSUM, bank-padded)
                smq = {}   # [P, 64] bf16 masked s, token-partition (SBUF)
                smt = {}   # [64, 128] bf16 masked s.T via PE transpose
                smt_sb = {}

                def sj_block(q):
                    # s[tok, j] += x-chunk.T @ A-chunk, rides the M-tile
                    sj[q] = psj.tile([P, 512], F32, tag="sj", name=f"sj{q}")
                    for k in range(KC):
                        nc.tensor.matmul(
                            sj[q][:, :NR],
                            x_sb[:, k, q * SUB : (q + 1) * SUB],
                            a_sb[:, k, :],
                            start=(k == 0),
                            stop=(k == KC - 1),
                        )

                def mask_q(q):
                    # rank-broadcast the per-adapter mask along r via a
                    # stride-0 AP: j = n*R + r
                    smq[q] = smqp.tile([P, NR], BF16, tag="smq", name=f"smq{q}")
                    m_bc = (
                        m_sb[:, s * N_SUB + q, :]
                        .unsqueeze(2)
                        .broadcast_to((P, N_ADAPT, R))
                    )
                    nc.vector.tensor_mul(
                        smq[q][:].rearrange("p (n r) -> p n r", n=N_ADAPT),
                        sj[q][:, :NR].rearrange("p (n r) -> p n r", n=N_ADAPT),
                        m_bc,
                    )

                def transpose_q(q):
                    smt[q] = pst.tile([NR, 1024], BF16, tag="smt", name=f"smt{q}")
                    nc.tensor.transpose(
                        smt[q][:, :SUB], smq[q][:], ident[:]
                    )

                def copy_q(q):
                    smt_sb[q] = smtp.tile(
                        [NR, SUB], BF16, tag="smtsb", name=f"smtsb{q}"
                    )
                    nc.vector.tensor_copy(smt_sb[q][:], smt[q][:, :SUB])

                def main_half(q, n, o_ps_h, skip=False):
                    ts = q * SUB
                    nsl = slice(n * 512, (n + 1) * 512)
                    for k in range(KC):
                        nc.tensor.matmul(
                            o_ps_h[:],
                            x_sb[:, k, ts : ts + SUB],
                            w_sb[:, k, nsl],
                            start=(k == 0),
                            stop=False,
                            skip_group_check=skip,
                        )

                def lora_half(q, n, o_ps_h, skip=False):
                    nsl = slice(n * 512, (n + 1) * 512)
                    nc.tensor.matmul(
                        o_ps_h[:],
                        smt_sb[q][:],
                        bt_sb[:, nsl],
                        start=False,
                        stop=True,
                        skip_group_check=skip,
                    )
                    o_sb = op.tile([P, 512], BF16, tag="o")
                    nc.vector.tensor_add(o_sb[:], o_ps_h[:], b_sb[:, nsl])
                    # the closing store rides the idle sync queue so its
                    # descriptor generation overlaps the scalar queue's
                    eng = nc.sync if (s == N_SUP - 1 and q == N_SUB - 1
                                      and n == NB - 1) else nc.scalar
                    eng.dma_start(out=out_r[s, q][:, nsl], in_=o_sb[:])

                if s == 0:
                    # Startup: k-outer across q0/q1 mains so each arriving
                    # (x-chunk, W-chunk) DMA pair unlocks PE work faster
                    # than the serialized preload stream delivers it.
                    ph01 = {}
                    for q in (0, 1, 2):
                        for n in range(NB):
                            ph01[q, n] = pso.tile(
                                [P, 512], F32, tag="ops", name=f"ops01_{q}_{n}"
                            )
                    for k in range(KC):
                        for q in (0, 1, 2):
                            for n in range(NB):
                                nsl = slice(n * 512, (n + 1) * 512)
                                nc.tensor.matmul(
                                    ph01[q, n][:],
                                    x_sb[:, k, q * SUB : (q + 1) * SUB],
                                    w_sb[:, k, nsl],
                                    start=(k == 0),
                                    stop=False,
                                    skip_group_check=True,
                                )
                    for q in range(N_SUB):
                        sj_block(q)
                        mask_q(q)
                    for q in range(N_SUB):
                        transpose_q(q)
                        copy_q(q)
                    for q in (0, 1, 2):
                        for n in range(NB):
                            lora_half(q, n, ph01[q, n], skip=True)
                    for q in (3,):
                        o_ps = {}
                        for n in range(NB):
                            o_ps[n] = pso.tile(
                                [P, 512], F32, tag="ops", name=f"ops0_{q}_{n}"
                            )
                            main_half(q, n, o_ps[n])
                        for n in range(NB):
                            lora_half(q, n, o_ps[n])
                else:
                    # Steady state: sj/transpose/copy for tile q run early,
                    # interleaved with the q-1/q main matmuls, so the LoRA-B
                    # matmul never waits on the DVE round trip.
                    o_ps = {}
                    sj_block(0)
                    mask_q(0)
                    o_ps[0, 0] = pso.tile([P, 512], F32, tag="ops", name="opsA")
                    main_half(0, 0, o_ps[0, 0])
                    sj_block(1)
                    mask_q(1)
                    transpose_q(0)
                    copy_q(0)
                    o_ps[0, 1] = pso.tile([P, 512], F32, tag="ops", name="opsB")
                    main_half(0, 1, o_ps[0, 1])
                    lora_half(0, 0, o_ps[0, 0])
                    lora_half(0, 1, o_ps[0, 1])
                    for q in (1, 2):
                        sj_block(q + 1)
                        mask_q(q + 1)
                        transpose_q(q)
                        copy_q(q)
                        for n in range(NB):
                            o_ps[q, n] = pso.tile(
                                [P, 512], F32, tag="ops", name=f"ops_{q}_{n}"
                            )
                            main_half(q, n, o_ps[q, n])
                        for n in range(NB):
                            lora_half(q, n, o_ps[q, n])
                    transpose_q(3)
                    copy_q(3)
                    for n in range(NB):
                        o_ps[3, n] = pso.tile(
                            [P, 512], F32, tag="ops", name=f"ops_3_{n}"
                        )
                        main_half(3, n, o_ps[3, n])
                    for n in range(NB):
                        lora_half(3, n, o_ps[3, n])

    nc.compile()
    return nc


_NC_CACHE = None


def _get_nc():
    global _NC_CACHE
    if _NC_CACHE is None:
        _NC_CACHE = build_bass()
    return _NC_CACHE


def make_in_maps(x, W, b, lora_A, lora_B, masks):
    x = np.ascontiguousarray(x, dtype=np.float32)
    W = np.ascontiguousarray(W, dtype=np.float32)
    b = np.ascontiguousarray(b, dtype=np.float32)
    lora_A = np.ascontiguousarray(lora_A, dtype=np.float32)
    lora_B = np.ascontiguousarray(lora_B, dtype=np.float32)
    masks = np.ascontiguousarray(masks, dtype=np.float32)

    x_flat = x.reshape(B * T, D_IN)
    A_flat = lora_A.reshape(NR, D_IN)
    B_flat = lora_B.transpose(1, 0, 2).reshape(D_OUT, NR)

    wt = np.ascontiguousarray(W.T.astype(NP_BF16))       # [D_IN, D_OUT]
    # packed [P, KC*NR]: per-partition contiguous 1KB rows (full DMA rate)
    at = np.ascontiguousarray(
        A_flat.T.astype(NP_BF16).reshape(KC, P, NR).transpose(1, 0, 2)
        .reshape(P, KC * NR)
    )
    btr = np.ascontiguousarray(B_flat.T.astype(NP_BF16))  # [NR, D_OUT]

    # per-token mask, token-partition layout [P, G*N_ADAPT]
    m_full = masks[..., 0].reshape(N_ADAPT, B * T) * np.float32(SCALING)

    in_maps = []
    for c in range(N_CORES):
        sl = slice(c * TOK, (c + 1) * TOK)
        mtok = np.ascontiguousarray(
            m_full[:, sl].T.astype(NP_BF16)             # [TOK, N]
            .reshape(G, P, N_ADAPT).transpose(1, 0, 2)  # [P, G, N]
            .reshape(P, G * N_ADAPT)
        )
        in_maps.append(
            {
                "xt": np.ascontiguousarray(x_flat[sl].astype(NP_BF16).T),
                "wt": wt,
                "at": at,
                "btr": btr,
                "bias": b.astype(NP_BF16),
                "mtok": mtok,
            }
        )
    return in_maps


def kernel(x, W, b, lora_A, lora_B, masks):
    nc = _get_nc()
    in_maps = make_in_maps(x, W, b, lora_A, lora_B, masks)
    res = run_bass_kernel_spmd(nc, in_maps, core_ids=list(range(N_CORES)))
    out = np.concatenate([r["out"] for r in res.results], axis=0)
    out = out.astype(np.float32).reshape(B, T, D_OUT)
    return out
